# revision 35
# baseline (speedup 1.0000x reference)
"""Trainium2 Bass kernel for FlaxSapama (Llama-style) attention block.

Strategy: tensor-parallel over heads across 8 NeuronCores.
Core m owns Q heads [4m..4m+4) and KV head m (GQA group of 4), plus the
matching slice of Wo rows. Each core computes a full [T, HIDDEN] partial
output (its heads' contribution through Wo); the host sums the 8 partials.

Per-core pipeline (all matmuls bf16 inputs, fp32 PSUM accumulation):
  1. QKV projections computed transposed: qT/kT [head_dim, tokens] via
     lhsT=W tile, rhs=hidden^T tile; RoPE applied on PSUM evacuation.
     V is DMA-transposed to [tokens, head_dim] tiles for the PV matmul.
  2. Attention with scores computed transposed: S^T[k,q] tiles; softmax
     denominators via ones-matmul accumulation in PSUM; causal masking via
     additive mask tiles on diagonal blocks only; exp on ScalarE with
     per-partition key-padding bias; 1/denominator broadcast across
     partitions via gpsimd.partition_broadcast.
  3. Output projection accumulating 4 head slices per PSUM tile.

Tiles are split per (head, batch) so the Tile scheduler can overlap the
three phases across batches.
"""

import math

import numpy as np
import ml_dtypes

import concourse.bacc as bacc
import concourse.tile as tile
import concourse.mybir as mybir
from concourse.bass_utils import run_bass_kernel_spmd

BF16 = mybir.dt.bfloat16
F32 = mybir.dt.float32
NP_BF16 = ml_dtypes.bfloat16

HIDDEN = 4096
N_HEADS = 32
N_KV = 8
HD = 128          # head dim
MAX_POS = 4096
B, S = 2, 2048
T = B * S         # 4096 tokens
NCORES = 8
HPC = N_HEADS // NCORES      # 4 q heads per core
KT = HIDDEN // 128           # 32 contraction tiles for projections
NB = T // 512                # 8 token blocks of 512
TT = T // 128                # 32 token tiles of 128
SKT = S // 128               # 16 k-pos tiles per batch
NEG = -1.0e9

_PROGRAM = None


def _build_program():
    nc = bacc.Bacc(None, target_bir_lowering=False)

    # all inputs partition-major so DMA runs are >=1KB contiguous per partition
    hst_d = nc.dram_tensor("hst", [128, KT, T], BF16, kind="ExternalInput")
    sin_d = nc.dram_tensor("sint", [128, T], BF16, kind="ExternalInput")
    cos_d = nc.dram_tensor("cost", [128, T], BF16, kind="ExternalInput")
    wq_d = nc.dram_tensor("wq", [128, HPC, KT, HD], BF16, kind="ExternalInput")
    wk_d = nc.dram_tensor("wk", [128, KT, HD], BF16, kind="ExternalInput")
    wv_d = nc.dram_tensor("wv", [128, KT, HD], BF16, kind="ExternalInput")
    wo_d = nc.dram_tensor("wo", [128, HPC, HIDDEN], BF16, kind="ExternalInput")
    masks_d = nc.dram_tensor("masks", [128, 4, 512], BF16, kind="ExternalInput")
    kb_d = nc.dram_tensor("kbias", [128, TT], F32, kind="ExternalInput")
    out_d = nc.dram_tensor("out", [TT, 128, HIDDEN], F32, kind="ExternalOutput")

    with tile.TileContext(nc) as tc:
        # one PSUM pool for the whole kernel: a shared "big" tag (6 banks) lets
        # consecutive phases overlap through slot recycling instead of
        # serializing on pool region reuse; "den" gets the other 2 banks
        with tc.tile_pool(name="qkv", bufs=1) as pool_qkv, \
             tc.tile_pool(name="psA", bufs=4, space="PSUM") as psA:
            # per-(head, batch) tiles so phases can overlap across batches
            qT = [[pool_qkv.tile([128, S], BF16, name=f"qT_{h}_{b}")
                   for b in range(B)] for h in range(HPC)]
            kTt = [pool_qkv.tile([128, S], BF16, name=f"kT_{b}") for b in range(B)]
            vt = [pool_qkv.tile([128, SKT, HD], BF16, name=f"v_{b}") for b in range(B)]
            aout = [[pool_qkv.tile([128, S], BF16, name=f"ao_{h}_{b}")
                     for b in range(B)] for h in range(HPC)]
            masks_sb = pool_qkv.tile([128, 4, 512], BF16)
            kb_sb = pool_qkv.tile([128, TT], F32)
            ones_sb = pool_qkv.tile([128, 1], BF16)
            nc.vector.memset(ones_sb, 1.0)

            # ---------------- Phase 1: QKV projections + RoPE ----------------
            with tc.tile_pool(name="p1w", bufs=1) as p1w, \
                 tc.tile_pool(name="p1h", bufs=2) as p1h, \
                 tc.tile_pool(name="p1t", bufs=2) as p1t:

                def load_hst(nb):
                    tok = slice(nb * 512, (nb + 1) * 512)
                    tiles = [p1h.tile([128, KT // 4, 512], BF16, tag=f"hst{q}",
                                      name=f"hst{q}_{nb}")
                             for q in range(4)]
                    for q in range(4):
                        nc.sync.dma_start(
                            out=tiles[q], in_=hst_d[:, q * 8:(q + 1) * 8, tok])
                    sin_t = p1h.tile([128, 512], BF16, tag="sin", name=f"sin_{nb}")
                    cos_t = p1h.tile([128, 512], BF16, tag="cos", name=f"cos_{nb}")
                    nc.sync.dma_start(out=sin_t, in_=sin_d[:, tok])
                    nc.sync.dma_start(out=cos_t, in_=cos_d[:, tok])
                    return tiles, sin_t, cos_t

                wq_sb = [p1w.tile([128, KT, HD], BF16, name=f"wq_{h}")
                         for h in range(HPC)]
                wk_sb = p1w.tile([128, KT, HD], BF16)
                wv_a = p1w.tile([128, 8, HD], BF16)
                wv_b = p1w.tile([128, KT - 8, HD], BF16)
                # DMA emission order matches first-chain consumption exactly
                nc.sync.dma_start(out=wv_a, in_=wv_d[:, 0:8, :])
                h0 = [p1h.tile([128, KT // 4, 512], BF16, tag=f"hst{q}",
                               name=f"hst{q}_0") for q in range(4)]
                nc.sync.dma_start(out=h0[0], in_=hst_d[:, 0:8, 0:512])
                nc.sync.dma_start(out=wv_b, in_=wv_d[:, 8:KT, :])
                nc.sync.dma_start(out=h0[1], in_=hst_d[:, 8:16, 0:512])
                nc.sync.dma_start(out=wk_sb, in_=wk_d[:, :, :])
                nc.sync.dma_start(out=h0[2], in_=hst_d[:, 16:24, 0:512])
                nc.sync.dma_start(out=h0[3], in_=hst_d[:, 24:32, 0:512])
                sin_0 = p1h.tile([128, 512], BF16, tag="sin", name="sin_0")
                cos_0 = p1h.tile([128, 512], BF16, tag="cos", name="cos_0")
                nc.sync.dma_start(out=sin_0, in_=sin_d[:, 0:512])
                nc.sync.dma_start(out=cos_0, in_=cos_d[:, 0:512])
                nb0_tiles = (h0, sin_0, cos_0)
                for h in range(HPC):
                    nc.sync.dma_start(out=wq_sb[h], in_=wq_d[:, h, :, :])
                nc.sync.dma_start(out=masks_sb, in_=masks_d[:, :, :])
                nc.sync.dma_start(out=kb_sb, in_=kb_d[:, :])

                for nb in range(NB):
                    b, qb = nb // 4, nb % 4
                    bsl = slice(qb * 512, (qb + 1) * 512)
                    hst_t, sin_t, cos_t = nb0_tiles if nb == 0 else load_hst(nb)

                    for m in ((5, 4, 0, 1, 2, 3) if nb < NB - 1 else (0, 1, 2, 3, 4, 5)):
                        psum = psA.tile([128, 512], F32, tag="big", name=f"pj_{nb}_{m}")
                        for kt in range(KT):
                            if m < HPC:
                                lhsT = wq_sb[m][:, kt, :]
                            elif m == HPC:
                                lhsT = wk_sb[:, kt, :]
                            elif kt < 8:
                                lhsT = wv_a[:, kt, :]
                            else:
                                lhsT = wv_b[:, kt - 8, :]
                            nc.tensor.matmul(psum[:, :], lhsT=lhsT,
                                             rhs=hst_t[kt // 8][:, kt % 8, :],
                                             start=(kt == 0), stop=(kt == KT - 1))
                        if m <= HPC:
                            # rope: out = x*cos + shift_half(x)*sin' (sign in sin')
                            # cross-half reads straight from PSUM (SB-SB ops need
                            # equal base partitions; PSUM+SB is exempt)
                            tmp = p1t.tile([128, 512], BF16, tag="tmp")
                            nc.vector.tensor_mul(tmp[0:64, :], psum[64:128, :], sin_t[0:64, :])
                            nc.vector.tensor_mul(tmp[64:128, :], psum[0:64, :], sin_t[64:128, :])
                            t2 = p1t.tile([128, 512], BF16, tag="t2")
                            nc.vector.tensor_mul(t2, psum[:, :], cos_t)
                            dest = qT[m][b][:, bsl] if m < HPC else kTt[b][:, bsl]
                            nc.vector.tensor_add(dest, t2, tmp)
                        else:
                            v_bf = p1t.tile([128, 512], BF16, tag="vbf")
                            nc.scalar.copy(out=v_bf, in_=psum[:, :])
                            for j in range(4):
                                nc.sync.dma_start_transpose(
                                    out=vt[b][:, qb * 4 + j, :],
                                    in_=v_bf[:, j * 128:(j + 1) * 128])

            # ---------------- Phases 2+3 ----------------
            # p3 pools open (and wo loads) before p2 pools so the wo DMA only
            # waits on phase-1 readers, not on all of phase 2
            with tc.tile_pool(name="p3c", bufs=1) as p3c, \
                 tc.tile_pool(name="p3t", bufs=4) as p3t, \
                 tc.tile_pool(name="p2c", bufs=1) as p2c, \
                 tc.tile_pool(name="p2t", bufs=4) as p2t:
                wo_sb = p3c.tile([128, HPC, HIDDEN], BF16)
                for h in range(HPC):
                    nc.sync.dma_start(out=wo_sb[:, h, :], in_=wo_d[:, h, :])
                def emit_s(b, h, qb, kt):
                    # diagonal tiles (kt-4qb = o > 0): columns < 128*o are
                    # fully causal-masked -> skip them entirely
                    o_off = kt - 4 * qb
                    c0 = 128 * o_off if o_off > 0 else 0
                    cs = slice(c0, 512)
                    s_ps = psA.tile([128, 512], F32, tag="big",
                                    name=f"s_{b}_{h}_{qb}_{kt}")
                    nc.tensor.matmul(
                        s_ps[:, cs],
                        lhsT=kTt[b][:, kt * 128:(kt + 1) * 128],
                        rhs=qT[h][b][:, qb * 512 + c0:(qb + 1) * 512],
                        start=True, stop=True)
                    if o_off >= 0:
                        nc.vector.tensor_add(s_ps[:, cs], s_ps[:, cs],
                                             masks_sb[:, o_off, cs])
                    p_bf = p2t.tile([128, 512], BF16, tag="p", bufs=10)
                    gk = b * SKT + kt
                    nc.scalar.activation(
                        out=p_bf[:, cs], in_=s_ps[:, cs],
                        func=mybir.ActivationFunctionType.Exp,
                        bias=kb_sb[:, gk:gk + 1], scale=1.0)
                    return b, h, qb, kt, p_bf, cs

                acc = {}
                tails = []

                def emit_dp(b, h, qb, kt, p_bf, cs):
                    nkt = 4 * (qb + 1)
                    if kt == 0:
                        acc[(b, h, qb)] = (
                            psA.tile([128, 512], F32, tag="obank", bufs=2,
                                     name=f"o_{b}_{h}_{qb}"),
                            psA.tile([1, 512], F32, tag="den", bufs=2,
                                     name=f"den_{b}_{h}_{qb}"))
                    o_ps, den_ps = acc[(b, h, qb)]
                    nc.tensor.matmul(den_ps[:, cs], lhsT=ones_sb[:, :],
                                     rhs=p_bf[:, cs],
                                     start=(kt == 0), stop=(kt == nkt - 1))
                    nc.tensor.matmul(o_ps[:, cs], lhsT=vt[b][:, kt, :],
                                     rhs=p_bf[:, cs],
                                     start=(kt == 0), stop=(kt == nkt - 1))
                    if kt == nkt - 1:
                        tails.append((b, h, qb))
                    elif kt == 2 and tails:
                        # deferred: run the previous qb's normalization on DVE
                        # after this qb's first mask-adds, not before them
                        emit_tail(*tails.pop(0))

                def emit_tail(b, h, qb):
                    o_ps, den_ps = acc.pop((b, h, qb))
                    qsl = slice(qb * 512, (qb + 1) * 512)
                    recip = p2t.tile([1, 512], F32, tag="recip")
                    nc.vector.reciprocal(recip, den_ps[:, :])
                    rb = p2t.tile([128, 512], F32, tag="rb")
                    nc.gpsimd.partition_broadcast(rb[:, :], recip[:, :])
                    nc.vector.tensor_mul(aout[h][b][:, qsl], o_ps[:, :], rb[:, :])

                # one software pipeline across the whole attention phase:
                # den/pv (and each qb's normalization tail) lag the scores
                # matmul by 5 iterations so PE never waits on the exp (ACT)
                pend = []
                for b in range(B):
                    for h in range(HPC):
                        for qb in range(4):
                            for kt in range(4 * (qb + 1)):
                                pend.append(emit_s(b, h, qb, kt))
                                if len(pend) > 7:
                                    emit_dp(*pend.pop(0))
                for args in pend:
                    emit_dp(*args)
                while tails:
                    emit_tail(*tails.pop(0))

            # ---------------- Phase 3: output projection ----------------
                oo_tags = (("big", None), ("big", None), ("obank", 2), ("den", 2))
                for tb in range(TT):
                    b, tloc = tb // SKT, tb % SKT
                    for ob in range(8):
                        tg, bf = oo_tags[(tb * 8 + ob) % 4]
                        o_ps3 = psA.tile([128, 512], F32, tag=tg, bufs=bf,
                                         name=f"oo_{tb}_{ob}")
                        for h in range(HPC):
                            nc.tensor.matmul(
                                o_ps3[:, :],
                                lhsT=aout[h][b][:, tloc * 128:(tloc + 1) * 128],
                                rhs=wo_sb[:, h, ob * 512:(ob + 1) * 512],
                                start=(h == 0), stop=(h == HPC - 1))
                        osb = p3t.tile([128, 512], F32, tag="osb", bufs=8)
                        if (tb * 8 + ob) % 2 == 0:
                            nc.scalar.copy(out=osb, in_=o_ps3[:, :])
                        else:
                            nc.vector.tensor_copy(out=osb, in_=o_ps3[:, :])
                        nc.sync.dma_start(out=out_d[tb, :, ob * 512:(ob + 1) * 512],
                                          in_=osb)
    nc.compile()
    return nc


def _rope_tables():
    freqs = np.einsum("i,j->ij", np.arange(MAX_POS),
                      1.0 / 10000 ** (np.arange(0, HD, 2) / HD)).astype("float32")
    emb = np.concatenate((freqs, freqs), axis=-1)  # [pos, HD]
    return np.sin(emb), np.cos(emb)


def _prep_inputs(hidden_states, attention_mask, position_ids, Wq, Wk, Wv, Wo):
    hs = np.ascontiguousarray(np.asarray(hidden_states, dtype=np.float32))
    am = np.asarray(attention_mask, dtype=np.float32)
    pid = np.asarray(position_ids).astype(np.int64).reshape(-1)

    sin, cos = _rope_tables()
    sinT = np.ascontiguousarray(sin[pid].T)   # [HD, T]
    cosT = np.ascontiguousarray(cos[pid].T)
    sinT[0:HD // 2] *= -1.0                    # fold rotate-half sign
    sin_in = sinT.astype(NP_BF16)
    cos_in = cosT.astype(NP_BF16)

    hsT = hs.reshape(T, HIDDEN).T                          # [HIDDEN, T]
    hst_in = np.ascontiguousarray(
        hsT.reshape(KT, 128, T).transpose(1, 0, 2)).astype(NP_BF16)  # [128, KT, T]

    # causal masks for diagonal blocks: allowed iff c >= 128*o + r
    r = np.arange(128)[:, None]
    c = np.arange(512)[None, :]
    masks = np.stack([np.where(c >= 128 * o + r, 0.0, NEG) for o in range(4)])
    masks = np.ascontiguousarray(masks.transpose(1, 0, 2)).astype(NP_BF16)  # [128,4,512]

    kb = np.where(am.reshape(-1) > 0, 0.0, NEG).astype(np.float32)
    kb_in = np.ascontiguousarray(kb.reshape(TT, 128).T)   # [128, TT]

    scale = 1.0 / math.sqrt(HD)
    Wq = np.asarray(Wq, dtype=np.float32) * scale
    Wk = np.asarray(Wk, dtype=np.float32)
    Wv = np.asarray(Wv, dtype=np.float32)
    Wo = np.asarray(Wo, dtype=np.float32)

    in_maps = []
    for m in range(NCORES):
        wq_m = np.ascontiguousarray(Wq[:, m * HPC * HD:(m + 1) * HPC * HD])
        wk_m = np.ascontiguousarray(Wk[:, m * HD:(m + 1) * HD])
        wv_m = np.ascontiguousarray(Wv[:, m * HD:(m + 1) * HD])
        wo_m = np.ascontiguousarray(Wo[m * HPC * HD:(m + 1) * HPC * HD, :])
        in_maps.append({
            "hst": hst_in,
            "sint": sin_in,
            "cost": cos_in,
            # [128, HPC, KT, HD]: partition-major, per-head blocked
            "wq": np.ascontiguousarray(
                wq_m.reshape(KT, 128, HPC, HD).transpose(1, 2, 0, 3)).astype(NP_BF16),
            "wk": np.ascontiguousarray(
                wk_m.reshape(KT, 128, HD).transpose(1, 0, 2)).astype(NP_BF16),
            "wv": np.ascontiguousarray(
                wv_m.reshape(KT, 128, HD).transpose(1, 0, 2)).astype(NP_BF16),
            "wo": np.ascontiguousarray(
                wo_m.reshape(HPC, 128, HIDDEN).transpose(1, 0, 2)).astype(NP_BF16),
            "masks": masks,
            "kbias": kb_in,
        })
    return in_maps


def get_program():
    global _PROGRAM
    if _PROGRAM is None:
        _PROGRAM = _build_program()
    return _PROGRAM


def kernel(**inputs):
    nc = get_program()
    in_maps = _prep_inputs(
        inputs["hidden_states"], inputs["attention_mask"], inputs["position_ids"],
        inputs["Wq"], inputs["Wk"], inputs["Wv"], inputs["Wo"])
    res = run_bass_kernel_spmd(nc, in_maps, core_ids=list(range(NCORES)))
    acc = np.zeros((TT, 128, HIDDEN), dtype=np.float32)
    for r in res.results:
        acc += r["out"]
    return acc.reshape(B, S, HIDDEN)
